# revision 4
# baseline (speedup 1.0000x reference)
"""Trainium2 Bass kernel for a dense transformer block (v2).

Problem: B=8, T=2048, DIM=384, 6 heads (hd=64), FFN hidden 768, causal
attention, RMSNorm (eps 1e-6), exact GELU, fp32 I/O.

Sharding: data-parallel over batch B=8 -> one batch element per NeuronCore,
no collectives. Each core runs the full block on its [2048, 384] slice.

v2 changes vs the 276us baseline (engine busy: PE 215 / ACT 177 / DVE 158):
  - Q^T is stored unpadded; QK matmuls contract K=64 and run as row-tiled
    PAIRS (tile_position (0,0)/(64,0)) so both heads of a feature chunk
    compute concurrently in different PSUM banks -> ~2x QK throughput.
    The pair writes the two halves of one [128, 1024] S tile, so one exp
    covers both heads.
  - PE transposes are regular bf16 matmuls against identity (N=128,
    FWL-eligible) instead of transpose-mode (which doesn't pipeline and
    doesn't engage HAM); 3 chunk transposes land in one PSUM bank and are
    evacuated with a single strided DVE copy into a fused feature-major
    tile (ht_all etc).
  - One activation-table set (natural_log_exp_and_others) covers the whole
    kernel except GELU: RMSNorm uses DVE square+accum (scalar_tensor_tensor)
    and batched rsqrt = Exp(-0.5*Ln(mean+eps)) in [128,4] groups. Only one
    table switch (to gelu) at the FFN tail.
  - Causal masking: full-width exp everywhere; diagonal k-tiles get one
    in-place DVE multiply per parity half against a precomputed 0/1 band
    (bf16, 2x mode) before the AV matmul. No memsets, no split exps.
  - AV keeps the ones-column normalizer trick (M=65, Z lands in PSUM row
    64 for free) but o is evacuated to SBUF immediately (one DVE copy) so
    the o PSUM banks recycle fast; the reciprocal/broadcast/multiply chain
    runs from SBUF in bf16 off the critical path.
  - Emission is chunk-pipelined (ch ascending): attention for chunk ch
    starts after only its own K/Q columns exist; x+o@wo, norm2 and the
    second transpose of chunk ch overlap attention of chunk ch+1.
  - PSUM budget: 2 (shared proj/transpose pool) + 4 (S double-buffer)
    + 2 (o_even/o_odd) = 8 banks during attention; FFN tail reuses the
    S/o banks for [128,1024] GELU tiles.
"""

import math
import sys

import ml_dtypes
import numpy as np

for _p in ("/opt/trn_rl_repo",):
    if _p not in sys.path:
        sys.path.append(_p)

import concourse.bacc as bacc
import concourse.bass as bass
import concourse.mybir as mybir
import concourse.tile as tile
from concourse.bass import ts
from concourse.bass_utils import run_bass_kernel_spmd
from concourse.masks import make_identity

F32 = mybir.dt.float32
BF16 = mybir.dt.bfloat16
AF = mybir.ActivationFunctionType
ALU = mybir.AluOpType

NCORES = 8
T, D, NH, HD, HDIM = 2048, 384, 6, 64, 768
P = 128
SLOT = HD + 1          # per-head V slot: [v_0..v_63, ones]
NT = T // P            # 16 token tiles
ND = D // P            # 3 feature chunks
NHT = HDIM // P        # 6 FFN hidden chunks
CH = 512               # Tq chunk width
NCH = T // CH          # 4
EPS = 1e-6
SCL = 1.0 / math.sqrt(HD)


def _body(tc, din, out_d):
    nc = tc.nc

    main_cm = tc.tile_pool(name="main", bufs=1)
    main = main_cm.__enter__()

    # ---- constants ----
    ident = main.tile([P, P], BF16, tag="ident", name="ident")
    make_identity(nc, ident[:])
    eps_t = main.tile([P, 1], F32, tag="eps", name="eps")
    nc.gpsimd.memset(eps_t[:], EPS)
    onesf = main.tile([P, P], F32, tag="onesf", name="onesf")
    nc.gpsimd.memset(onesf[:], 1.0)
    ones_bf = main.tile([1, P], BF16, tag="ones", name="ones")
    nc.vector.tensor_copy(ones_bf[:], onesf[0:1, :])
    # band[k, c] = 1 iff c - k >= CH (0/1 mask for causal diagonals)
    bandf = main.tile([P, 2 * CH], F32, tag="bandf", name="bandf")
    nc.gpsimd.memset(bandf[:], 1.0)
    nc.gpsimd.affine_select(out=bandf[:], in_=bandf[:],
                            compare_op=ALU.is_ge, fill=0.0,
                            base=-CH, channel_multiplier=-1,
                            pattern=[[1, 2 * CH]])
    band = main.tile([P, 2 * CH], BF16, tag="band", name="band")
    nc.vector.tensor_copy(band[:], bandf[:])

    s1 = main.tile([P, NT], F32, tag="s1", name="s1")
    s1i = main.tile([P, NT], F32, tag="s1i", name="s1i")
    s2 = main.tile([P, NT], F32, tag="s2", name="s2")
    s2i = main.tile([P, NT], F32, tag="s2i", name="s2i")

    # ---- big feature-major tensors (single tiles; chunk c = cols c*T..) ----
    ht = main.tile([P, ND * T], BF16, tag="ht", name="ht")
    qt = main.tile([P, ND * T], BF16, tag="qt", name="qt")
    kt = main.tile([P, ND * T], BF16, tag="kt", name="kt")
    ot = main.tile([P, ND * T], BF16, tag="ot", name="ot")
    h2t = main.tile([P, ND * T], BF16, tag="h2t", name="h2t")
    gt = main.tile([P, NHT * T], BF16, tag="gt", name="gt")

    x_tiles = [main.tile([P, D], F32, tag=f"x{j}", name=f"x{j}")
               for j in range(NT)]
    vaug = [main.tile([P, NH * SLOT], BF16, tag=f"va{j}", name=f"va{j}")
            for j in range(NT)]

    # ---- weights ----
    wq_s = [main.tile([P, D], BF16, tag=f"wq{c}", name=f"wq{c}") for c in range(ND)]
    wk_s = [main.tile([P, D], BF16, tag=f"wk{c}", name=f"wk{c}") for c in range(ND)]
    wv_s = [main.tile([P, D], BF16, tag=f"wv{c}", name=f"wv{c}") for c in range(ND)]
    wo_s = [main.tile([P, D], BF16, tag=f"wo{c}", name=f"wo{c}") for c in range(ND)]
    fw1_s = [main.tile([P, HDIM], BF16, tag=f"f1{c}", name=f"f1{c}") for c in range(ND)]
    fw2_s = [main.tile([P, D], BF16, tag=f"f2{c}", name=f"f2{c}") for c in range(NHT)]
    b1_s = main.tile([P, NHT], F32, tag="b1", name="b1")
    b2_row = main.tile([1, D], BF16, tag="b2", name="b2")

    for j in range(NT):
        nc.sync.dma_start(x_tiles[j][:], din["x"][ts(j, P), :])
    for c in range(ND):
        nc.sync.dma_start(wq_s[c][:], din["wq"][ts(c, P), :])
        nc.sync.dma_start(wk_s[c][:], din["wk"][ts(c, P), :])
        nc.sync.dma_start(wv_s[c][:], din["wv"][ts(c, P), :])
        nc.sync.dma_start(wo_s[c][:], din["wo"][ts(c, P), :])
        nc.sync.dma_start(fw1_s[c][:], din["fw1"][ts(c, P), :])
    for c in range(NHT):
        nc.sync.dma_start(fw2_s[c][:], din["fw2"][ts(c, P), :])
    nc.sync.dma_start(b1_s[:], din["fb1"].rearrange("(a b) -> b a", b=P))
    nc.sync.dma_start(b2_row[:], din["fb2"].rearrange("(a b) -> a b", a=1))

    for j in range(NT):
        nc.vector.tensor_copy(
            vaug[j][:].rearrange("p (h e) -> p h e", h=NH)[:, :, HD:SLOT],
            onesf[:, 0:NH].rearrange("p (h e) -> p h e", e=1))

    # ---- scratch pools ----
    pscr_cm = tc.tile_pool(name="scr", bufs=3)
    pscr = pscr_cm.__enter__()
    patt_cm = tc.tile_pool(name="att", bufs=3)
    patt = patt_cm.__enter__()
    pnrm_cm = tc.tile_pool(name="nrm", bufs=2)
    pnrm = pnrm_cm.__enter__()
    pout_cm = tc.tile_pool(name="out", bufs=3)
    pout = pout_cm.__enter__()

    pj_cm = tc.tile_pool(name="pj", bufs=2, space="PSUM")
    pj = pj_cm.__enter__()

    def norm_and_transpose(js, s_acc, s_inv, dst):
        """RMSNorm stats on DVE, batched rsqrt on ACT, scale + PE transpose."""
        for j in js:
            sq = pscr.tile([P, D], F32, tag="sq", name="sq")
            nc.vector.scalar_tensor_tensor(
                sq[:], x_tiles[j][:], 1.0, x_tiles[j][:],
                op0=ALU.mult, op1=ALU.mult,
                accum_out=s_acc[:, j : j + 1])
        j0 = js[0]
        n = len(js)
        lnt = pscr.tile([P, 4], F32, tag="ln", name="ln")
        nc.scalar.activation(lnt[:, 0:n], s_acc[:, j0 : j0 + n], AF.Ln,
                             scale=1.0 / D, bias=eps_t[:, 0:1])
        nc.scalar.activation(s_inv[:, j0 : j0 + n], lnt[:, 0:n], AF.Exp,
                             scale=-0.5)
        for j in js:
            hb = pscr.tile([P, D], BF16, tag="hb", name="hb")
            nc.vector.tensor_scalar_mul(hb[:], x_tiles[j][:],
                                        s_inv[:, j : j + 1])
            tp = pj.tile([P, CH], F32, tag="pj", name="tp")
            for c in range(ND):
                nc.tensor.matmul(tp[:, ts(c, P)], hb[:, ts(c, P)], ident[:],
                                 start=True, stop=True)
            nc.vector.tensor_copy(
                dst[:].rearrange("p (c t) -> p c t", c=ND)[:, :, ts(j, P)],
                tp[:, 0:D].rearrange("p (c t) -> p c t", c=ND))

    # ---- phase 0: norm1 + transpose for chunk 0 ----
    norm_and_transpose(range(4), s1, s1i, ht)

    psS_cm = tc.tile_pool(name="psS", bufs=2, space="PSUM")
    psS = psS_cm.__enter__()
    psO_cm = tc.tile_pool(name="psO", bufs=1, space="PSUM")
    psO = psO_cm.__enter__()

    for ch in range(NCH):
        js = range(4 * ch, 4 * ch + 4)
        # K^T / Q^T columns for this chunk
        for dt in range(ND):
            ps = pj.tile([P, CH], F32, tag="pj", name="kq")
            for c in range(ND):
                nc.tensor.matmul(
                    ps[:], wk_s[c][:, ts(dt, P)], ht[:, c * T + ch * CH :
                                                      c * T + ch * CH + CH],
                    start=(c == 0), stop=(c == ND - 1))
            nc.vector.tensor_copy(kt[:, dt * T + ch * CH :
                                     dt * T + ch * CH + CH], ps[:])
        for dt in range(ND):
            ps = pj.tile([P, CH], F32, tag="pj", name="kq")
            for c in range(ND):
                nc.tensor.matmul(
                    ps[:], wq_s[c][:, ts(dt, P)], ht[:, c * T + ch * CH :
                                                      c * T + ch * CH + CH],
                    start=(c == 0), stop=(c == ND - 1))
            nc.vector.tensor_copy(qt[:, dt * T + ch * CH :
                                     dt * T + ch * CH + CH], ps[:])
        # V (token-major, into per-head slots)
        for j in js:
            ps = pj.tile([P, CH], F32, tag="pj", name="v")
            for c in range(ND):
                nc.tensor.matmul(
                    ps[:, 0:D], ht[:, c * T + j * P : c * T + (j + 1) * P],
                    wv_s[c][:], start=(c == 0), stop=(c == ND - 1))
            nc.vector.tensor_copy(
                vaug[j][:].rearrange("p (h e) -> p h e", h=NH)[:, :, 0:HD],
                ps[:, 0:D].rearrange("p (h e) -> p h e", h=NH))

        # ---- attention for chunk ch ----
        ntk = 4 * (ch + 1)
        for dt in range(ND):
            o_e = psO.tile([P, CH], F32, tag="oe", name="oe")
            o_o = psO.tile([P, CH], F32, tag="oo", name="oo")
            for k in range(ntk):
                s_ps = psS.tile([P, 2 * CH], F32, tag="s", name="s")
                nc.tensor.matmul(
                    s_ps[:, 0:CH],
                    kt[0:HD, dt * T + k * P : dt * T + (k + 1) * P],
                    qt[0:HD, dt * T + ch * CH : dt * T + ch * CH + CH],
                    start=True, stop=True, tile_position=(0, 0))
                nc.tensor.matmul(
                    s_ps[:, CH : 2 * CH],
                    kt[HD:P, dt * T + k * P : dt * T + (k + 1) * P],
                    qt[HD:P, dt * T + ch * CH : dt * T + ch * CH + CH],
                    start=True, stop=True, tile_position=(HD, 0))
                p_sb = patt.tile([P, 2 * CH], BF16, tag="p", name="p")
                nc.scalar.activation(p_sb[:], s_ps[:], AF.Exp, scale=SCL)
                b = k - 4 * ch
                if b >= 0:
                    for par in range(2):
                        nc.vector.tensor_mul(
                            p_sb[:, par * CH : (par + 1) * CH],
                            p_sb[:, par * CH : (par + 1) * CH],
                            band[:, CH - P * b : 2 * CH - P * b])
                nc.tensor.matmul(
                    o_e[0:SLOT, :],
                    vaug[k][:, (2 * dt) * SLOT : (2 * dt + 1) * SLOT],
                    p_sb[:, 0:CH],
                    start=(k == 0), stop=(k == ntk - 1))
                nc.tensor.matmul(
                    o_o[0:SLOT, :],
                    vaug[k][:, (2 * dt + 1) * SLOT : (2 * dt + 2) * SLOT],
                    p_sb[:, CH : 2 * CH],
                    start=(k == 0), stop=(k == ntk - 1))
            # evacuate + normalize both heads (o/Z; Z sits in PSUM row 64)
            for par, o_ps in ((0, o_e), (1, o_o)):
                o_sb = pnrm.tile([P, CH], BF16, tag="osb", name="osb")
                nc.vector.tensor_copy(o_sb[0:HD, :], o_ps[0:HD, :])
                zf = pnrm.tile([P, CH], F32, tag="zf", name="zf")
                nc.vector.tensor_copy(zf[0:1, :], o_ps[HD : HD + 1, :])
                nc.vector.reciprocal_approx_fast(zf[0:1, :], zf[0:1, :])
                zb = pnrm.tile([P, CH], BF16, tag="zb", name="zb")
                nc.vector.tensor_copy(zb[0:1, :], zf[0:1, :])
                zbb = pnrm.tile([P, CH], BF16, tag="zbb", name="zbb")
                nc.gpsimd.partition_broadcast(zbb[0:HD, :], zb[0:1, :])
                nc.vector.tensor_mul(o_sb[0:HD, :], o_sb[0:HD, :],
                                     zbb[0:HD, :])
                hp = par * HD
                nc.sync.dma_start(
                    ot[hp : hp + HD, dt * T + ch * CH : dt * T + ch * CH + CH],
                    o_sb[0:HD, :])

        # ---- norm1 + transpose for the NEXT chunk (fills the pipeline) ----
        if ch < NCH - 1:
            norm_and_transpose(range(4 * ch + 4, 4 * ch + 8), s1, s1i, ht)

        # ---- x2 = x + o @ wo for this chunk ----
        for j in js:
            ps = pj.tile([P, CH], F32, tag="pj", name="xo")
            for dt in range(ND):
                nc.tensor.matmul(
                    ps[:, 0:D], ot[:, dt * T + j * P : dt * T + (j + 1) * P],
                    wo_s[dt][:], start=(dt == 0), stop=(dt == ND - 1))
            nc.vector.tensor_add(x_tiles[j][:], ps[:, 0:D], x_tiles[j][:])

        # ---- norm2 + transpose for this chunk ----
        norm_and_transpose(js, s2, s2i, h2t)

    psO_cm.__exit__(None, None, None)
    psS_cm.__exit__(None, None, None)

    # ---- FFN tail ----
    psF_cm = tc.tile_pool(name="psF", bufs=3, space="PSUM")
    psF = psF_cm.__enter__()
    for hti in range(NHT):
        for h2 in range(2):
            g_ps = psF.tile([P, 2 * CH], F32, tag="g", name="g")
            for m in range(2):
                col = (2 * h2 + m) * CH
                for c in range(ND):
                    nc.tensor.matmul(
                        g_ps[:, ts(m, CH)], fw1_s[c][:, ts(hti, P)],
                        h2t[:, c * T + col : c * T + col + CH],
                        start=(c == 0), stop=(c == ND - 1))
            nc.scalar.activation(
                gt[:, hti * T + h2 * 2 * CH : hti * T + (h2 + 1) * 2 * CH],
                g_ps[:], AF.Gelu, bias=b1_s[:, hti : hti + 1])
    for j in range(NT):
        ps = pj.tile([P, CH], F32, tag="pj", name="f2")
        for c in range(NHT):
            nc.tensor.matmul(
                ps[:, 0:D], gt[:, c * T + j * P : c * T + (j + 1) * P],
                fw2_s[c][:], start=(c == 0), stop=False)
        nc.tensor.matmul(ps[:, 0:D], ones_bf[0:1, :], b2_row[0:1, :],
                         start=False, stop=True)
        o_t = pout.tile([P, D], F32, tag="ot", name="otl")
        nc.vector.tensor_add(o_t[:], ps[:, 0:D], x_tiles[j][:])
        nc.sync.dma_start(out_d[ts(j, P), :], o_t[:])

    psF_cm.__exit__(None, None, None)
    pj_cm.__exit__(None, None, None)
    pout_cm.__exit__(None, None, None)
    pnrm_cm.__exit__(None, None, None)
    patt_cm.__exit__(None, None, None)
    pscr_cm.__exit__(None, None, None)
    main_cm.__exit__(None, None, None)


_CACHE = {}


def _build():
    if "nc" in _CACHE:
        return _CACHE["nc"]
    nc = bacc.Bacc("TRN2", target_bir_lowering=False, debug=False)
    din = {}
    for name, shape, dt_ in (
        ("x", [T, D], F32), ("wq", [D, D], BF16), ("wk", [D, D], BF16),
        ("wv", [D, D], BF16), ("wo", [D, D], BF16), ("fw1", [D, HDIM], BF16),
        ("fb1", [HDIM], F32), ("fw2", [HDIM, D], BF16), ("fb2", [D], BF16),
    ):
        din[name] = nc.dram_tensor(name, shape, dt_, kind="ExternalInput").ap()
    out_d = nc.dram_tensor("out", [T, D], F32, kind="ExternalOutput").ap()
    with tile.TileContext(nc) as tc:
        _body(tc, din, out_d)
    nc.compile()
    _CACHE["nc"] = nc
    return nc


def run(inputs: dict, trace: bool = False):
    """Run on 8 cores; returns (output [8,T,D], BassKernelResults)."""
    nc = _build()
    x = np.ascontiguousarray(inputs["x"], dtype=np.float32)
    ln1 = np.asarray(inputs["ln1_w"], dtype=np.float32)
    ln2 = np.asarray(inputs["ln2_w"], dtype=np.float32)
    shared = {
        "wq": (ln1[:, None] * np.asarray(inputs["wq"], np.float32)).astype(ml_dtypes.bfloat16),
        "wk": (ln1[:, None] * np.asarray(inputs["wk"], np.float32)).astype(ml_dtypes.bfloat16),
        "wv": (ln1[:, None] * np.asarray(inputs["wv"], np.float32)).astype(ml_dtypes.bfloat16),
        "wo": np.asarray(inputs["wo"], np.float32).astype(ml_dtypes.bfloat16),
        "fw1": (ln2[:, None] * np.asarray(inputs["ff_w1"], np.float32)).astype(ml_dtypes.bfloat16),
        "fb1": np.asarray(inputs["ff_b1"], np.float32),
        "fw2": np.asarray(inputs["ff_w2"], np.float32).astype(ml_dtypes.bfloat16),
        "fb2": np.asarray(inputs["ff_b2"], np.float32).astype(ml_dtypes.bfloat16),
    }
    shared = {k: np.ascontiguousarray(v) for k, v in shared.items()}
    in_maps = [dict(shared, x=np.ascontiguousarray(x[c])) for c in range(NCORES)]
    res = run_bass_kernel_spmd(nc, in_maps, list(range(NCORES)), trace=trace)
    out = np.stack([res.results[c]["out"] for c in range(NCORES)], axis=0)
    return out, res


def kernel(**inputs) -> np.ndarray:
    out, _ = run(inputs, trace=False)
    return out
